# revision 12
# baseline (speedup 1.0000x reference)
"""Trainium2 Bass kernel for nn_BinarizedCifar10MLP.

Strategy: data-parallel over the batch (8192/8 = 1024 rows per core), with
feature-major ("transposed") activation layout [features, batch] on device so
no on-device transposes are needed anywhere.  BatchNorm batch statistics are
all-reduced across the 8 cores (3 tiny AllReduces of [128,64] fp32).

Precision scheme (reference is fp32):
  - L1 (x @ sign(W1).T): x is split losslessly on the host into fp16 hi + lo
    pieces (x == hi + lo exactly); each piece is matmul'd against sign(W1) in
    fp16 at full PE rate and accumulated in the same fp32 PSUM group.  Since
    sign(W1) in fp16 is exact and fp16 products vs +-1 are exact, the result
    carries only fp32-accumulation noise.
  - L2/L3: activations and weights are exact +-1 in fp16; sums of +-1 over
    2048 terms accumulate exactly in fp32 PSUM (integers < 2^24).
  - L4: y3/W4 in fp16 (2^-12 relative), log-softmax in fp32.
"""

import sys

sys.path.insert(0, "/opt/trn_rl_repo")

import numpy as np

B, D, H, C = 8192, 3 * 32 * 32, 2048, 10
EPS = 1e-5
NCORES = 8
BS = B // NCORES          # batch rows per core
KD = D // 128             # 24 k-tiles over input dim
KH = H // 128             # 16 k-tiles over hidden dim
NB = BS // 512            # 2 free-dim chunks of 512

_CACHE = {}
USE_3D_WDMA = True
M_LIMIT = KH  # debug knob: number of m-tiles per layer


def _build(stage=7):
    import concourse.bacc as bacc
    import concourse.mybir as mybir
    import concourse.tile as tile

    F32 = mybir.dt.float32
    F16 = mybir.dt.float16
    ACT = mybir.ActivationFunctionType
    ALU = mybir.AluOpType
    RG = [list(range(NCORES))]

    nc = bacc.Bacc("TRN2", target_bir_lowering=False, debug=False, num_devices=NCORES)

    # ---- I/O ----
    xhi_d = nc.dram_tensor("xT_hi", [D, BS], F16, kind="ExternalInput").ap()
    xlo_d = nc.dram_tensor("xT_lo", [D, BS], F16, kind="ExternalInput").ap()
    w1t_d = nc.dram_tensor("W1T", [D, H], F32, kind="ExternalInput").ap()
    w2t_d = nc.dram_tensor("W2T", [H, H], F32, kind="ExternalInput").ap()
    w3t_d = nc.dram_tensor("W3T", [H, H], F32, kind="ExternalInput").ap()
    w4t_d = nc.dram_tensor("W4T", [H, C], F32, kind="ExternalInput").ap()
    cons_d = {}
    for name in ("b1", "g1", "bt1", "b2", "g2", "bt2", "b3", "g3", "bt3"):
        cons_d[name] = nc.dram_tensor("c_" + name, [128, KH], F32, kind="ExternalInput").ap()
    b4_d = nc.dram_tensor("c_b4", [16, 1], F32, kind="ExternalInput").ap()
    out_d = nc.dram_tensor("outT", [C, BS], F32, kind="ExternalOutput").ap()

    wl_d = {1: w1t_d, 2: w2t_d, 3: w3t_d}
    kl = {1: KD, 2: KH, 3: KH}          # contraction k-tiles per layer

    with tile.TileContext(nc) as tc:
        with (
            tc.tile_pool(name="pconst", bufs=1) as pconst,
            tc.tile_pool(name="pstat", bufs=1) as pstat,
            tc.tile_pool(name="plog", bufs=1) as plog,
            tc.tile_pool(name="pscr", bufs=3) as pscr,
            tc.tile_pool(name="pw32", bufs=2) as pw32,
            tc.tile_pool(name="pw16", bufs=2) as pw16,
            tc.tile_pool(name="ph", bufs=1) as ph,
            tc.tile_pool(name="pb", bufs=1) as pb,
            tc.tile_pool(name="pa", bufs=1) as pa,
            tc.tile_pool(name="ppsum", bufs=8, space="PSUM") as ppsum,
            tc.tile_pool(name="pdram", bufs=6, space="DRAM") as pdram,
        ):
            # ---- constants ----
            cons = {}
            for name in cons_d:
                t = pconst.tile([128, KH], F32, tag="c_" + name)
                nc.sync.dma_start(t[:], cons_d[name])
                cons[name] = t
            b4s = pconst.tile([16, 1], F32, tag="b4")
            nc.sync.dma_start(b4s[:], b4_d)
            ones10 = pconst.tile([16, 1], F32, tag="ones10")
            nc.vector.memset(ones10[:], 1.0)
            w4st = pconst.tile([128, C * KH], F32, tag="w4st")
            for k in range(KH):
                nc.sync.dma_start(w4st[:, k * C:(k + 1) * C], w4t_d[k * 128:(k + 1) * 128, :])
            w4f = pconst.tile([128, C * KH], F16, tag="w4f")
            nc.vector.tensor_copy(w4f[:], w4st[:])

            # ---- load x.T pieces (fp16 hi/lo) ----
            xhi = pa.tile([128, KD * BS], F16, tag="pa")
            xlo = pb.tile([128, KD * BS], F16, tag="pb")
            for k in range(KD):
                nc.sync.dma_start(xhi[:, k * BS:(k + 1) * BS], xhi_d[k * 128:(k + 1) * 128, :])
                nc.sync.dma_start(xlo[:, k * BS:(k + 1) * BS], xlo_d[k * 128:(k + 1) * 128, :])

            parts = {}
            glob = {}

            def dense_layer(l, rhs_hi, rhs_lo):
                """h_l.T = sign(W_l).T-block matmuls; returns SBUF h tile + parts."""
                K = kl[l]
                wt_d = wl_d[l]
                h_t = ph.tile([128, KH * BS], F32, tag="ph")
                parts_l = pstat.tile([128, 64], F32, tag=f"parts{l}")
                bias_t = cons[f"b{l}"]
                n_kg = K // 8  # kgroups of 8 k-tiles
                for m in range(M_LIMIT):
                    w16 = pw16.tile([128, K * 128], F16, tag="w16")
                    for kg in range(n_kg):
                        wst = pw32.tile([128, 1024], F32, tag="w32")
                        if USE_3D_WDMA:
                            src = wt_d[kg * 1024:(kg + 1) * 1024, m * 128:(m + 1) * 128]
                            nc.sync.dma_start(
                                wst[:].rearrange("p (j c) -> p j c", j=8),
                                src.rearrange("(j p) c -> p j c", p=128),
                            )
                        else:
                            for j in range(8):
                                k = kg * 8 + j
                                nc.sync.dma_start(
                                    wst[:, j * 128:(j + 1) * 128],
                                    wt_d[k * 128:(k + 1) * 128, m * 128:(m + 1) * 128],
                                )
                        nc.scalar.activation(w16[:, kg * 1024:(kg + 1) * 1024], wst[:], ACT.Sign)
                    for n in range(NB):
                        ps = ppsum.tile([128, 512], F32, tag="ps")
                        for k in range(K):
                            lhsT = w16[:, k * 128:(k + 1) * 128]
                            sl = slice(k * BS + n * 512, k * BS + n * 512 + 512)
                            nc.tensor.matmul(ps[:], lhsT, rhs_hi[:, sl], start=(k == 0), stop=(rhs_lo is None and k == K - 1))
                            if rhs_lo is not None:
                                nc.tensor.matmul(ps[:], lhsT, rhs_lo[:, sl], start=False, stop=(k == K - 1))
                        hs = h_t[:, m * BS + n * 512: m * BS + n * 512 + 512]
                        col = 2 * m + n
                        nc.scalar.activation(hs, ps[:], ACT.Identity, bias=bias_t[:, m:m + 1],
                                             scale=1.0, accum_out=parts_l[:, col:col + 1])
                        scr = pscr.tile([128, BS], F32, tag="scr")
                        nc.scalar.activation(scr[:, :512], hs, ACT.Square,
                                             accum_out=parts_l[:, 32 + col:32 + col + 1])
                parts[l] = parts_l
                return h_t

            def bn_stats(l):
                """AllReduce parts -> per-feature scale rp (=g*rsqrt(v+eps)) and bias c."""
                arin = pdram.tile([128, 64], F32, tag=f"arin{l}")
                arout = pdram.tile([128, 64], F32, tag=f"arout{l}")
                nc.sync.dma_start(arin[:], parts[l][:])
                nc.gpsimd.collective_compute(
                    "AllReduce", ALU.add, replica_groups=RG,
                    ins=[arin.opt()], outs=[arout.opt()])
                g_t = pstat.tile([128, 64], F32, tag=f"glob{l}")
                nc.sync.dma_start(g_t[:], arout[:])
                glob[l] = g_t

                def st(tag):
                    return pstat.tile([128, KH], F32, name=f"{tag}{l}", tag=f"{tag}{l}")

                sg, qg, m1, msq, m1sq, v, sq, r, rp, mt, c = (
                    st(x) for x in ("sg", "qg", "m1", "msq", "m1sq", "v", "sq", "r", "rp", "mt", "c"))
                nc.vector.tensor_reduce(sg[:], g_t[:, 0:32].rearrange("p (m n) -> p m n", n=2),
                                        axis=mybir.AxisListType.X, op=ALU.add)
                nc.vector.tensor_reduce(qg[:], g_t[:, 32:64].rearrange("p (m n) -> p m n", n=2),
                                        axis=mybir.AxisListType.X, op=ALU.add)
                nc.vector.tensor_scalar_mul(m1[:], sg[:], 1.0 / B)
                nc.vector.tensor_scalar_mul(msq[:], qg[:], 1.0 / B)
                nc.vector.tensor_tensor(m1sq[:], m1[:], m1[:], op=ALU.mult)
                nc.vector.tensor_tensor(v[:], msq[:], m1sq[:], op=ALU.subtract)
                nc.vector.tensor_scalar_add(v[:], v[:], EPS)
                nc.scalar.activation(sq[:], v[:], ACT.Sqrt)
                nc.vector.reciprocal(r[:], sq[:])
                nc.vector.tensor_tensor(rp[:], cons[f"g{l}"][:], r[:], op=ALU.mult)
                nc.vector.tensor_tensor(mt[:], m1[:], rp[:], op=ALU.mult)
                nc.vector.tensor_tensor(c[:], cons[f"bt{l}"][:], mt[:], op=ALU.subtract)
                return rp, c

            def debug_out(src_ap, cast=False):
                """DMA a [C, BS] f32 view to out for stage bisection."""
                if cast:
                    t = pscr.tile([128, BS], F32, tag="scr", name="dbgcast")
                    nc.vector.tensor_copy(t[:C, :], src_ap)
                    src_ap = t[:C, :]
                nc.sync.dma_start(out_d[:], src_ap)

            # ===== Layer 1 =====
            h1 = dense_layer(1, xhi, xlo)
            if stage == 1:
                debug_out(h1[:C, :BS])
            if stage >= 2:
                rp1, c1 = bn_stats(1)
                a2 = pa.tile([128, KH * BS], F16, tag="pa")   # reuses xT_hi slot
                for k in range(KH):
                    nc.scalar.activation(a2[:, k * BS:(k + 1) * BS], h1[:, k * BS:(k + 1) * BS],
                                         ACT.Sign, bias=c1[:, k:k + 1], scale=rp1[:, k:k + 1])
                if stage == 2:
                    debug_out(a2[:C, :BS], cast=True)

            if stage >= 3:
                # ===== Layer 2 =====
                h2 = dense_layer(2, a2, None)
                rp2, c2 = bn_stats(2)
                a3 = pb.tile([128, KH * BS], F16, tag="pb")   # reuses xT_lo slot
                for k in range(KH):
                    nc.scalar.activation(a3[:, k * BS:(k + 1) * BS], h2[:, k * BS:(k + 1) * BS],
                                         ACT.Sign, bias=c2[:, k:k + 1], scale=rp2[:, k:k + 1])
                if stage == 3:
                    debug_out(a3[:C, :BS], cast=True)

            if stage >= 4:
                # ===== Layer 3 =====
                h3 = dense_layer(3, a3, None)
                rp3, c3 = bn_stats(3)
                y3 = pa.tile([128, KH * BS], F16, tag="pa")   # reuses a2 slot
                for k in range(KH):
                    scr = pscr.tile([128, BS], F32, tag="scr")
                    nc.scalar.activation(scr[:], h3[:, k * BS:(k + 1) * BS],
                                         ACT.Identity, bias=c3[:, k:k + 1], scale=rp3[:, k:k + 1])
                    nc.vector.tensor_scalar(out=y3[:, k * BS:(k + 1) * BS], in0=scr[:],
                                            scalar1=-1.0, scalar2=1.0, op0=ALU.max, op1=ALU.min)
                if stage == 4:
                    debug_out(y3[:C, :BS], cast=True)

            if stage >= 5:
                # ===== Layer 4 + log-softmax =====
                logits = plog.tile([16, BS], F32, tag="logits")
                for n in range(NB):
                    ps4 = ppsum.tile([128, 512], F32, tag="ps")
                    for k in range(KH):
                        nc.tensor.matmul(ps4[:C, :], w4f[:, k * C:(k + 1) * C],
                                         y3[:, k * BS + n * 512: k * BS + n * 512 + 512],
                                         start=(k == 0), stop=(k == KH - 1))
                    nc.scalar.activation(logits[:C, n * 512:(n + 1) * 512], ps4[:C, :],
                                         ACT.Identity, bias=b4s[:C, :], scale=1.0)
                if stage == 5:
                    debug_out(logits[:C, :])

            if stage >= 6:
                e_t = pscr.tile([128, BS], F32, tag="scr")
                nc.scalar.activation(e_t[:C, :], logits[:C, :], ACT.Exp)
                lse = pscr.tile([128, BS], F32, tag="scr")
                for n in range(NB):
                    ps5 = ppsum.tile([128, 512], F32, tag="ps")
                    nc.tensor.matmul(ps5[:1, :], ones10[:C, :], e_t[:C, n * 512:(n + 1) * 512],
                                     start=True, stop=True)
                    nc.scalar.activation(lse[:1, n * 512:(n + 1) * 512], ps5[:1, :], ACT.Ln)
                lse10 = pscr.tile([128, BS], F32, tag="scr")
                nc.gpsimd.partition_broadcast(lse10[:C, :], lse[:1, :], channels=C)
                outs = plog.tile([16, BS], F32, tag="outs")
                nc.vector.tensor_tensor(outs[:C, :], logits[:C, :], lse10[:C, :], op=ALU.subtract)
                nc.sync.dma_start(out_d[:], outs[:C, :])

    nc.compile()
    return nc


def _prep_inputs(x, W1, b1, g1, bt1, W2, b2, g2, bt2, W3, b3, g3, bt3, W4, b4):
    """Host-side sharding + layout prep (pure layout/permutation + lossless split)."""
    def as32(a):
        return np.ascontiguousarray(np.asarray(a, dtype=np.float32))

    x = as32(x)
    shared = {
        "W1T": np.ascontiguousarray(as32(W1).T),
        "W2T": np.ascontiguousarray(as32(W2).T),
        "W3T": np.ascontiguousarray(as32(W3).T),
        "W4T": np.ascontiguousarray(as32(W4).T),
    }
    for name, v in (("b1", b1), ("g1", g1), ("bt1", bt1), ("b2", b2), ("g2", g2),
                    ("bt2", bt2), ("b3", b3), ("g3", g3), ("bt3", bt3)):
        shared["c_" + name] = np.ascontiguousarray(as32(v).reshape(KH, 128).T)
    b4p = np.zeros((16, 1), np.float32)
    b4p[:C, 0] = as32(b4).reshape(-1)
    shared["c_b4"] = b4p

    in_maps = []
    for c in range(NCORES):
        xT = np.ascontiguousarray(x[c * BS:(c + 1) * BS].T)     # [D, BS]
        hi = xT.astype(np.float16)
        lo = (xT - hi.astype(np.float32)).astype(np.float16)    # exact residual fits fp16
        m = dict(shared)
        m["xT_hi"] = hi
        m["xT_lo"] = lo
        in_maps.append(m)
    return in_maps


def kernel(**inputs) -> np.ndarray:
    from concourse.bass_utils import run_bass_kernel_spmd

    if "nc" not in _CACHE:
        _CACHE["nc"] = _build()
    nc = _CACHE["nc"]
    in_maps = _prep_inputs(**inputs)
    res = run_bass_kernel_spmd(nc, in_maps, list(range(NCORES)))
    out = np.concatenate([res.results[c]["outT"].T for c in range(NCORES)], axis=0)
    return out.astype(np.float32)
